# revision 8
# baseline (speedup 1.0000x reference)
"""Bass/Trainium2 kernel for nn_DCDicl (DSBlock forward).

Per sample: Q = Unfold_pad4(x)^T @ Unfold_pad4(x) (+ a*I), P = U^T Yz (+ a*d),
D = cho_solve(Q, P).  The dominant FLOPs (the 25.6 GFLOP/sample Gram matrix)
run on 8 NeuronCores: data-parallel over the 4 samples x 2 halves of the
10000-row contraction.  Host does the unfold layout, the tiny P (64 MFLOP),
and the 1600x1600 solve.
"""

import sys

import numpy as np

if "/opt/trn_rl_repo" not in sys.path:
    sys.path.append("/opt/trn_rl_repo")

N, C_IN, C_OUT, H, W, DS = 4, 64, 4, 96, 96, 5
K = C_IN * DS * DS            # 1600
KP = 1664                     # 13 * 128, padded column count
ROWS = 100 * 100              # unfold output positions
HALF = 5120                   # 40 * 128 rows per core (2 halves of 10000, padded)
KCH = HALF // 128             # 40 k-chunks
NT = 256                      # n-tile width (psum free dim)
N_NT = KP // NT               # 6.5 -> 7 handled below
M_MT = KP // 128              # 13 m-tiles

_CACHED = {}


def _build_nc():
    """Raw-Bass double-buffered Gram kernel.

    All input DMAs increment ONE shared dma semaphore (order-independent
    cumulative count), so every consumer needs at most 2 sync waits —
    the hardware per-instruction wait-command limit that Tile's scheduler
    blew through for this pattern.
    """
    from contextlib import ExitStack

    import concourse.bass as bass
    import concourse.mybir as mybir

    nc = bass.Bass()
    u_dram = nc.dram_tensor("u", [HALF, KP], mybir.dt.float32, kind="ExternalInput")
    q_dram = nc.dram_tensor("q", [KP, KP], mybir.dt.float32, kind="ExternalOutput")

    n_nt = (KP + NT - 1) // NT  # 7; last n-tile is 128 wide
    m_his = [min(2 * (n + 1), M_MT) for n in range(n_nt)]
    # schedule tables: per block b -> (n, m, nt, dma count before PE may run)
    blocks = []
    din = 0
    for n in range(n_nt):
        din += KCH  # rhs strip chunks
        for m in range(m_his[n]):
            din += KCH  # lhs chunks
            blocks.append((n, m, min(NT, KP - n * NT), din))
    nblocks = len(blocks)
    cumb = np.cumsum([0] + m_his)  # blocks completed before strip n

    with ExitStack() as ctx:
        rhs_b = [
            ctx.enter_context(nc.sbuf_tensor(f"rhs{i}", [128, KCH, NT], mybir.dt.float32))
            for i in range(2)
        ]
        lhs_b = [
            ctx.enter_context(nc.sbuf_tensor(f"lhs{i}", [128, KCH, 128], mybir.dt.float32))
            for i in range(2)
        ]
        stage = [
            ctx.enter_context(nc.sbuf_tensor(f"stage{i}", [128, NT], mybir.dt.float32))
            for i in range(2)
        ]
        psum = [
            ctx.enter_context(nc.psum_tensor(f"ps{i}", [128, NT], mybir.dt.float32))
            for i in range(2)
        ]
        dma_sem = ctx.enter_context(nc.semaphore("dma_sem"))
        pe_sem = ctx.enter_context(nc.semaphore("pe_sem"))
        ve_sem = ctx.enter_context(nc.semaphore("ve_sem"))
        gp_sem = ctx.enter_context(nc.semaphore("gp_sem"))
        block = ctx.enter_context(nc.Block())

        @block.sync
        def _(sync):
            b = 0
            for n in range(n_nt):
                nt = min(NT, KP - n * NT)
                if n >= 2:  # rhs buffer reused from strip n-2
                    sync.wait_ge(pe_sem, int(cumb[n - 1]))
                for c in range(KCH):
                    sync.dma_start(
                        out=rhs_b[n % 2][:, c, :nt],
                        in_=u_dram[c * 128:(c + 1) * 128, n * NT:n * NT + nt],
                    ).then_inc(dma_sem, 16)
                for m in range(m_his[n]):
                    if b >= 2:  # lhs buffer reused from block b-2
                        sync.wait_ge(pe_sem, b - 1)
                    for c in range(KCH):
                        sync.dma_start(
                            out=lhs_b[b % 2][:, c, :],
                            in_=u_dram[c * 128:(c + 1) * 128, m * 128:(m + 1) * 128],
                        ).then_inc(dma_sem, 16)
                    b += 1

        @block.tensor
        def _(tensor):
            for b, (n, m, nt, din_b) in enumerate(blocks):
                tensor.wait_ge(dma_sem, 16 * din_b)
                if b >= 2:  # psum reused after copy of block b-2
                    tensor.wait_ge(ve_sem, b - 1)
                for c in range(KCH):
                    ins = nc.tensor.matmul(
                        psum[b % 2][:, :nt],
                        lhs_b[b % 2][:, c, :],
                        rhs_b[n % 2][:, c, :nt],
                        start=(c == 0),
                        stop=(c == KCH - 1),
                    )
                ins.then_inc(pe_sem, 1)

        @block.vector
        def _(vector):
            for b, (n, m, nt, _) in enumerate(blocks):
                vector.wait_ge(pe_sem, b + 1)
                if b >= 2:  # stage buffer reused after out-DMA of b-2
                    vector.wait_ge(gp_sem, 16 * (b - 1))
                nc.vector.tensor_copy(
                    stage[b % 2][:, :nt], psum[b % 2][:, :nt]
                ).then_inc(ve_sem, 1)

        @block.gpsimd
        def _(gpsimd):
            for b, (n, m, nt, _) in enumerate(blocks):
                gpsimd.wait_ge(ve_sem, b + 1)
                gpsimd.dma_start(
                    out=q_dram[m * 128:(m + 1) * 128, n * NT:n * NT + nt],
                    in_=stage[b % 2][:, :nt],
                ).then_inc(gp_sem, 16)

    return nc


def _unfold(x1):
    """x1: [C_in, H, W] -> U [10000, 1600] with U[(g,w'),(i,ph,pw)] = xpad[...]"""
    from numpy.lib.stride_tricks import sliding_window_view

    xp2 = np.pad(x1, ((0, 0), (4, 4), (4, 4)))
    sw = sliding_window_view(xp2, (DS, DS), axis=(1, 2))  # [C,100,100,5,5]
    return np.ascontiguousarray(
        sw.transpose(1, 2, 0, 3, 4).reshape(ROWS, K), dtype=np.float32
    )


def kernel(x, d, y, alpha, reg):
    from concourse import bass_utils

    x = np.asarray(x, dtype=np.float32)
    d = np.asarray(d, dtype=np.float32)
    y = np.asarray(y, dtype=np.float32)
    alpha = np.asarray(alpha, dtype=np.float32)
    reg = np.asarray(reg, dtype=np.float32)

    if "nc" not in _CACHED:
        _CACHED["nc"] = _build_nc()
    nc = _CACHED["nc"]

    # Host: build padded unfold matrices and shard over 8 cores.
    in_maps = []
    Us = []
    for s in range(N):
        U = _unfold(x[s, 0])  # [10000, 1600]
        Us.append(U)
        Up = np.zeros((2 * HALF, KP), dtype=np.float32)
        Up[:ROWS, :K] = U
        in_maps.append({"u": np.ascontiguousarray(Up[:HALF])})
        in_maps.append({"u": np.ascontiguousarray(Up[HALF:])})

    res = bass_utils.run_bass_kernel_spmd(nc, in_maps, core_ids=list(range(8)))
    outs = res.results

    a = alpha.reshape(N) * H * W * float(reg[0]) / (DS * DS * C_IN)

    out = np.empty((N, C_OUT, C_IN, DS, DS), dtype=np.float32)
    for s in range(N):
        Qp = outs[2 * s]["q"] + outs[2 * s + 1]["q"]
        Qu = np.triu(Qp[:K, :K].astype(np.float64))
        Q = Qu + np.triu(Qp[:K, :K].astype(np.float64), 1).T
        Q += a[s] * np.eye(K)

        # P = U^T Yz  (+ a * d): Yz is y embedded at offset (2,2) in the 100x100 grid
        Yz = np.zeros((100, 100, C_OUT), dtype=np.float32)
        Yz[2:2 + H, 2:2 + W, :] = y[s, :, 0].transpose(1, 2, 0)
        P = Us[s].T.astype(np.float64) @ Yz.reshape(ROWS, C_OUT).astype(np.float64)
        P += a[s] * d[s].transpose(1, 2, 3, 0).reshape(K, C_OUT)

        D = np.linalg.solve(Q, P)  # SPD, kappa ~ 6
        out[s] = D.reshape(C_IN, DS, DS, C_OUT).transpose(3, 0, 1, 2)
    return out
